# revision 65
# baseline (speedup 1.0000x reference)
"""TRN2 Bass kernel for nn_AttentionStoreProcessor (dense transformer attention).

Full (unsharded) inputs in, full output out. Internally:
  - CAPE rotation + softmax scale folded into Wq/Wk on host (exact linear
    algebra, per-frame 4x4 block-diagonal right-multiply).
  - Balanced 2.5-head sharding: each core owns 2 full heads (A, B) and one
    half head (C, one query half). Odd cores get their hs token-halves
    swapped on host (attention is permutation-invariant over keys) so one
    SPMD program covers both half assignments; host un-swaps their output.
  - hs arrives pre-transposed from host as one fp8 [ch, tok] tensor that
    serves the q/k projections (fp8 moving x bf16 stationary weights) and
    the v projection (DoubleRow channel pairs are just [:, 2kp:2kp+2, :]).
  - q/k weights in bf16 (accuracy-critical); v projection, scores,
    probs*V and the output projection all in fp8e4m3 with DoubleRow perf
    mode (0.5 PE cycles/row). Scores broadcast the pair dim (0-stride, x2
    result folded into Wq); PV and the output projection contract real
    pairs (two kt tiles / both outT planes per matmul).
  - softmax: max-free exp (scores are O(10)); constant bias -4.5 keeps
    exp in fp8 range; denominators via a ones-column appended to V; the
    per-query reciprocal is broadcast with a K=1 matmul.
  - Activation engine runs only the exps plus a few tail copies (exp time
    is the roofline for this shard); PSUM evacuations go to DVE (gpsimd
    cannot access PSUM on real hardware). Emission order software-pipelines
    the in-order engines: projections drip between attention units on a
    due-date worklist, PV matmuls and normalize chains are deferred past
    the scores they would otherwise block, and the final query-half is
    normalized in pieces so the last out-proj tiles start immediately.
  - residual, bias and the cross-core partial-sum reduction happen on host.
"""
import numpy as np
import ml_dtypes
from contextlib import ExitStack

import concourse.bacc as bacc
import concourse.mybir as mybir
import concourse.tile as tile
from concourse.bass_utils import run_bass_kernel_spmd

F32 = mybir.dt.float32
F32R = mybir.dt.float32r
BF16 = mybir.dt.bfloat16
F8 = mybir.dt.float8e4
NPBF16 = ml_dtypes.bfloat16
NPF8 = ml_dtypes.float8_e4m3
AF = mybir.ActivationFunctionType
DR = mybir.MatmulPerfMode.DoubleRow

HEADS = 20
N_CORES = 8
S = 2048  # tokens
D = 1280  # channels
HD = 64  # head dim
L = 1024  # tokens per frame
KT = D // 128  # 10 contraction tiles for projections
KP = KT // 2  # 5 channel-pair tiles
TOKT = S // 128  # 16 token tiles
EXP_BIAS = -4.5

_CACHED_NC = None


def _build_nc():
    nc = bacc.Bacc("TRN2", debug=False, num_devices=N_CORES)

    # hs^T fp8, chunk-major: [8 token-chunks, 128, KT, 256] so each chunk
    # is one contiguous-per-partition DMA. Serves the q/k projections as fp8
    # moving data (bf16 stationary weights) and the v projection as DoubleRow
    # channel pairs via [:, ci, 2kp:2kp+2, :].
    hstb_d = nc.dram_tensor("hstb", [8 * 128, KT * 256], F8, kind="ExternalInput").ap()
    # q-side + C-head weights bf16: 4 blocks [128, KT*128] = (t0:g0,g2, t1:g0,g2)
    wg_d = nc.dram_tensor("wg", [128, 4 * KT * 128], BF16, kind="ExternalInput").ap()
    # A/B k-side weights fp8 pair layout [128, t, KP, 2, 128]
    wk_d = nc.dram_tensor("wk8", [128, 2 * KP * 2 * 128], F8, kind="ExternalInput").ap()
    # v weights fp8 pair layout [128, KP, 2, 256]
    wv_d = nc.dram_tensor("wv8", [128, KP * 2 * 256], F8, kind="ExternalInput").ap()
    # out-proj weights fp8 [128, 2, D]: plane 0 = (A|0), plane 1 = (C|0);
    # B is applied from oT1tmp with its own base-0 weights wob
    wo_d = nc.dram_tensor("wo8", [128, 2 * D], F8, kind="ExternalInput").ap()
    wob_d = nc.dram_tensor("wob8", [64, D], F8, kind="ExternalInput").ap()
    out = nc.dram_tensor("out", [S, D], BF16, kind="ExternalOutput").ap()

    out_r = out.rearrange("(n p) d -> n p d", p=128)

    with (
        tile.TileContext(nc) as tc,
        ExitStack() as ctx,
        nc.allow_low_precision(reason="fp8/bf16 used deliberately; tolerance 2e-2"),
    ):
        persist = ctx.enter_context(tc.tile_pool(name="persist", bufs=1))
        hstb_pool = tc.alloc_tile_pool(name="hstb", bufs=1)
        u_pool = tc.alloc_tile_pool(name="u", bufs=12)
        rc_pool = tc.alloc_tile_pool(name="rc", bufs=3)
        ob_pool = tc.alloc_tile_pool(name="ob", bufs=6)

        pj_psum = tc.alloc_tile_pool(name="pj", bufs=1, space="PSUM")
        sc_psum = tc.alloc_tile_pool(name="sc", bufs=2, space="PSUM")
        pv_psum = tc.alloc_tile_pool(name="pv", bufs=3, space="PSUM")

        # ---- persistent tiles ----
        ones_sb = persist.tile([128, 64], BF16, tag="ones")
        expbias = persist.tile([128, 1], F32, tag="expbias")
        # per-kt stride padded 195 -> 208: dual-fp8 ldweights requires the
        # pair-dim step to be even and 16B-aligned
        v195 = persist.tile([128, TOKT, 208], F8, tag="v195")
        QA = persist.tile([128, S], F8, tag="QA")  # rows 0:64 qA, 64:128 qB
        KA = persist.tile([128, S], F8, tag="KA")  # rows 0:64 kA, 64:128 kB
        QK2 = persist.tile([128, S], F8, tag="QK2")  # rows 0:64 qC, 64:128 kC
        QB2 = persist.tile([128, S], F8, tag="QB2")  # rows 64:128 <- qC (shifted)
        outTall = persist.tile([128, 2, S], F8, tag="outTall")
        oT1tmp = persist.tile([64, S], F8, tag="oT1tmp")
        wg_sb = persist.tile([128, 4, KT, 128], BF16, tag="wg")
        wk_sb = persist.tile([128, 2, KP, 2, 128], F8, tag="wk8")
        wv_sb = persist.tile([128, KP, 2, 256], F8, tag="wv8")
        wo_sb = persist.tile([128, 2, D], F8, tag="wo8")
        wob_sb = persist.tile([64, D], F8, tag="wob8")

        nc.gpsimd.memset(ones_sb[:], 1.0)
        nc.gpsimd.memset(expbias[:], EXP_BIAS)
        # ones columns of v_ext (col 65h+64 = 1.0); plane-1 zeros of outTall
        v195_h = v195[:, :, 0:195].rearrange("p n (h x) -> p n h x", h=3)
        nc.vector.memset(v195_h[:, :, :, 64:65], 1.0)
        nc.vector.memset(outTall[:, 1, :], 0.0)
        nc.vector.memset(outTall[64:128, 0, :], 0.0)

        # ---- input DMAs ----
        # scalar queue: weights only (all dispatched before the first exp);
        # sync queue: hs tiles, chunk-pipelined.
        def wg_dma(g):
            nc.scalar.dma_start(
                wg_sb[:, g, :, :],
                wg_d[:, g * KT * 128 : (g + 1) * KT * 128].rearrange(
                    "p (k m) -> p k m", k=KT
                ),
            )

        wg_dma(0)
        nc.scalar.dma_start(
            wk_sb[:],
            wk_d.rearrange("p (t k two m) -> p t k two m", t=2, k=KP, two=2),
        )
        nc.scalar.dma_start(
            wv_sb[:], wv_d.rearrange("p (k two m) -> p k two m", k=KP, two=2)
        )
        for g in range(1, 4):
            wg_dma(g)
        nc.scalar.dma_start(wo_sb[:], wo_d.rearrange("p (two d) -> p two d", two=2))
        nc.scalar.dma_start(wob_sb[:], wob_d)
        hstb = hstb_pool.tile([128, 8, KT, 256], F8, tag="hstb")
        hstb_src = hstb_d.rearrange("(c p) (k s) -> p c k s", p=128, k=KT)
        nc.sync.dma_start(hstb[:, 0, 0:5, :], hstb_src[:, 0, 0:5, :])
        nc.sync.dma_start(hstb[:, 0, 5:10, :], hstb_src[:, 0, 5:10, :])
        for ci in range(1, 8):
            nc.sync.dma_start(hstb[:, ci, :, :], hstb_src[:, ci, :, :])

        # ---- projection work units (emitted interleaved with attention) ----
        # one psum bank, manually double-buffered by alternating 256-col
        # halves so an item's accumulation never waits the previous item's
        # DVE evacuation (range-level deps keep the halves independent)
        proj_ps = pj_psum.tile([128, 512], F32, tag="pj")
        pj_flip = [0]

        def pj_half():
            pj_flip[0] ^= 1
            o = pj_flip[0] * 256
            return proj_ps[:, o : o + 256]

        def emit_qk(ci, g):
            t = ci // 4  # core-local frame
            qs = slice(ci * 256, (ci + 1) * 256)
            dest = (QA, KA, QK2)[g]
            pp = pj_half()
            if g == 1:
                # A/B k-side in fp8 DoubleRow over channel pairs (4x fewer
                # PE cycles; scores requantize k to fp8 anyway)
                for kp in range(KP):
                    nc.tensor.matmul(
                        pp,
                        wk_sb[:, t, kp, :, :],
                        hstb[:, ci, 2 * kp : 2 * kp + 2, :],
                        start=(kp == 0),
                        stop=(kp == KP - 1),
                        perf_mode=DR,
                    )
            else:
                for k in range(KT):
                    nc.tensor.matmul(
                        pp,
                        wg_sb[:, t * 2 + (0 if g == 0 else 1), k, :],
                        hstb[:, ci, k, :],
                        start=(k == 0),
                        stop=(k == KT - 1),
                    )
            nc.vector.tensor_copy(dest[:, qs], pp)
            if g == 2:
                # shift qC (QK2 rows 0:64) to QB2 rows 64:128 (same base as
                # kC); gpsimd queue: cheap SEQ, runs in the pre-outproj window
                nc.gpsimd.dma_start(QB2[64:128, qs], QK2[0:64, qs])

        qk_half_state = {}

        def emit_qk_half(ci, g, part):
            # bf16 q/k chunk in two 5-ktile halves so the PE insert between
            # attention units stays below one exp time
            t = ci // 4
            if part == 0:
                pp = pj_half()
                qk_half_state[(ci, g)] = pp
            else:
                pp = qk_half_state.pop((ci, g))
            for k in range(part * 5, part * 5 + 5):
                nc.tensor.matmul(
                    pp,
                    wg_sb[:, t * 3 + g, k, :],
                    hstb[:, ci, k, :],
                    start=(k == 0),
                    stop=(k == KT - 1),
                )
            if part == 1:
                qs = slice(ci * 256, (ci + 1) * 256)
                nc.vector.tensor_copy((QA, KA, QK2)[g][:, qs], pp)
                if g == 2:
                    nc.gpsimd.dma_start(QB2[64:128, qs], QK2[0:64, qs])

        def emit_v(n):
            # v projection for token tile n (fp8 DoubleRow over channel pairs)
            vp = pj_half()
            for kp in range(KP):
                nc.tensor.matmul(
                    vp,
                    hstb[:, n // 2, 2 * kp : 2 * kp + 2, (n % 2) * 128 : (n % 2) * 128 + 128],
                    wv_sb[:, kp, :, :],
                    start=(kp == 0),
                    stop=(kp == KP - 1),
                    perf_mode=DR,
                )
            nc.vector.tensor_copy(
                v195_h[:, n, :, 0:64],
                vp[:, 0:192].rearrange("p (h x) -> p h x", h=3),
            )

        def head_ops(h):
            # (kT source, rows, qT source, rows) -- both at the same base
            if h == 0:
                return KA, slice(0, 64), QA, slice(0, 64)
            if h == 1:
                return KA, slice(64, 128), QA, slice(64, 128)
            return QK2, slice(64, 128), QB2, slice(64, 128)

        def brc(ap, n):
            # insert broadcast pair dim: [64, n] -> [64, 2, n], stride 0
            return ap.unsqueeze(1).broadcast_to((64, 2, n))

        def score_exp_pv(h, qh, half, ktp, pvt, name, first=False):
            # one attention unit: two broadcast-pair score matmuls into a
            # [128,1024] psum tile, one wide exp (amortizes the ACT access
            # penalty), one DoubleRow PV matmul contracting both kt tiles
            ksrc, krows, qsrc, qrows = head_ops(h)
            qcol = qh * 1024 + half * 512
            u2 = u_pool.tile([128, 2, 512], F8, tag="u", name=f"u{name}")
            sc = sc_psum.tile([128, 1024], F32, tag="sc", name=f"sc{name}")
            if first:
                # very first unit: 256-column sub-scores so the first exp
                # needs only q-chunk 0; the chunk-1 projection is emitted
                # between the halves and hides behind the first exp
                for qsub in range(2):
                    if qsub == 1:
                        emit_qk(1, 0)
                    for r in range(2):
                        kt = 2 * ktp + r
                        nc.tensor.matmul(
                            sc[:, r * 512 + qsub * 256 : r * 512 + qsub * 256 + 256],
                            brc(ksrc[krows, kt * 128 : (kt + 1) * 128], 128),
                            brc(qsrc[qrows, qcol + qsub * 256 : qcol + qsub * 256 + 256], 256),
                            start=True,
                            stop=True,
                            perf_mode=DR,
                        )
                    nc.scalar.activation(
                        u2[:, :, qsub * 256 : (qsub + 1) * 256],
                        sc[:].rearrange("p (two n) -> p two n", two=2)[
                            :, :, qsub * 256 : (qsub + 1) * 256
                        ],
                        AF.Exp,
                        bias=expbias[:],
                    )
            else:
                for r in range(2):
                    kt = 2 * ktp + r
                    nc.tensor.matmul(
                        sc[:, r * 512 : (r + 1) * 512],
                        brc(ksrc[krows, kt * 128 : (kt + 1) * 128], 128),
                        brc(qsrc[qrows, qcol : qcol + 512], 512),
                        start=True,
                        stop=True,
                        perf_mode=DR,
                    )
                nc.scalar.activation(
                    u2[:],
                    sc[:].rearrange("p (two n) -> p two n", two=2),
                    AF.Exp,
                    bias=expbias[:],
                )

            def pv():
                nc.tensor.matmul(
                    pvt,
                    v195[:, 2 * ktp : 2 * ktp + 2, 65 * h : 65 * h + 65],
                    u2[:],
                    start=(ktp == 0),
                    stop=(ktp == TOKT // 2 - 1),
                    perf_mode=DR,
                )

            return pv

        def norm_dest(h, c0, c1):
            if h == 0:
                return outTall[0:64, 0, c0:c1]
            if h == 1:
                return oT1tmp[:, c0:c1]
            return outTall[0:64, 1, c0:c1]

        def normalize_parts(h, qh, sub, pvt, pieces=1, tail=False):
            # returns closures: [recip, then per piece: bc+mul]
            q0 = qh * 1024 + sub * 512
            nm = f"{h}_{qh}_{sub}"
            rc = rc_pool.tile([65, 512], BF16, tag="rc", name=f"rc{nm}")
            w = 512 // pieces

            def recip():
                nc.vector.reciprocal(rc[64:65, :], pvt[64:65, :])

            def piece(i):
                def fn():
                    ps = slice(i * w, (i + 1) * w)
                    bc = sc_psum.tile([64, w], F32, tag="sc", name=f"bc{nm}_{i}")
                    nc.tensor.matmul(
                        bc[:], ones_sb[64:65, :], rc[64:65, ps], start=True, stop=True
                    )
                    # HW allows only one PSUM input per tensor-tensor op
                    bcs = rc_pool.tile([64, w], F32, tag="bcs", name=f"bcs{nm}_{i}")
                    if tail:
                        nc.scalar.copy(bcs[:], bc[:])  # ACT idle past last exp
                    else:
                        nc.vector.tensor_copy(bcs[:], bc[:])
                    nc.vector.tensor_mul(
                        norm_dest(h, q0 + i * w, q0 + (i + 1) * w), pvt[0:64, ps], bcs[:]
                    )
                return fn

            return [recip] + [piece(i) for i in range(pieces)]

        def emit_op(n):
            # output projection for token tile n (one DoubleRow matmul per
            # 512-wide Wo chunk contracts both outT planes = 192 features)
            ts = slice(n * 128, (n + 1) * 128)
            ob = ob_pool.tile([128, D], BF16, tag="ob", name=f"ob{n}")
            for dc, (off, w) in enumerate(((0, 512), (512, 512), (1024, 256))):
                op = pv_psum.tile([128, 512], F32, tag="pv", name=f"op{n}_{dc}")
                nc.tensor.matmul(
                    op[:, 0:w],
                    outTall[:, :, ts],
                    wo_sb[:, :, off : off + w],
                    start=True,
                    stop=False,
                    perf_mode=DR,
                )
                nc.tensor.matmul(
                    op[:, 0:w],
                    oT1tmp[:, ts],
                    wob_sb[:, off : off + w],
                    start=False,
                    stop=True,
                )
                if n >= 12 and dc != 1:
                    # tail: ACT is past its last exp and otherwise idle
                    nc.scalar.copy(ob[:, off : off + w], op[:, 0:w])
                else:
                    nc.vector.tensor_copy(ob[:, off : off + w], op[:, 0:w])
            deng = nc.gpsimd if n >= 12 and n % 2 == 1 else nc.sync
            deng.dma_start(out_r[n], ob[:])

        # ---- interleaved emission: attention unit stream + projection drip ----
        # pending projection units with due-unit indices (due = unit index in
        # the first section's stream, before which the item must be emitted;
        # PE executes nearly in emission order, so due-dates track data needs:
        # v tiles 2k,2k+1 before PV at ktp=k; k-side chunks before their kt
        # range; q-sides/g2 tails before the sections that read them)
        pending = [
            (1, emit_qk, (1, 1)),
            (1, emit_qk, (0, 2)),
            (2, emit_qk, (1, 2)),
        ]
        # v tiles 2k,2k+1 feed PV at ktp=k (unit 3k); k-side 256-token chunk
        # ci feeds kt tiles 2ci,2ci+1 (A/B at unit 3ci, C one unit later)
        pending += [(max(3 * k + 2, 1), emit_v, (2 * k,)) for k in range(8)]
        pending += [(max(3 * k + 2, 1), emit_v, (2 * k + 1,)) for k in range(8)]
        pending += [(3 * ci - 4, emit_qk, (ci, 1)) for ci in range(2, 8)]
        pending += [(3 * ci, emit_qk, (ci, 2)) for ci in range(2, 8)]
        # q-side: chunks 2,3 are query-half1 of qh0 (due unit 24); 4..7 are qh1
        pending += [(13, emit_qk, (2, 0)), (17, emit_qk, (3, 0))]
        pending += [(26 + 3 * j, emit_qk, (4 + j, 0)) for j in range(4)]
        # out-projection tiles drip through the qh1 attention stream as their
        # outTall columns complete (tiles 0..7 after qh0, 8..11 after
        # qh1-half0); 12..15 are the tail
        pending += [(53 + 2 * j, emit_op, (j,)) for j in range(8)]
        pending += [(69 + 3 * j, emit_op, (8 + j,)) for j in range(4)]
        pending.sort(key=lambda e: e[0])
        pi = 0

        def drip(unit):
            nonlocal pi
            while pi < len(pending) and pending[pi][0] <= unit:
                _, fn, args = pending[pi]
                fn(*args)
                pi += 1

        # phase A: the minimum before the first score (qA, kA chunk 0; qA
        # needs both 256-chunks of the first 512 query columns), then the
        # C-head projections + shifts overlap the first A/B exps
        for ci, g in ((0, 0), (0, 1)):
            emit_qk(ci, g)

        unit = 0
        deferred_pv = []
        deferred_post = []
        for qh in range(2):
            heads = (0, 1, 2) if qh == 0 else (0, 1)
            for half in range(2):
                last = qh == 1 and half == 1
                pvt = {
                    h: pv_psum.tile(
                        [65, 512], F32, tag="pv", name=f"pv{qh}_{half}_{h}"
                    )
                    for h in heads
                }
                # C lags A/B by one kt-pair so its q-shift DMA (issued by the
                # dripped g2 projection) is never on the critical PE path
                seq = []
                for ktp in range(TOKT // 2):
                    for h in heads[:2]:
                        seq.append((h, ktp))
                    if len(heads) > 2 and ktp >= 1:
                        seq.append((2, ktp - 1))
                if len(heads) > 2:
                    seq.append((2, TOKT // 2 - 1))
                for h, ktp in seq:
                    # previous section's normalize/shift closures drip
                    # one per unit so they never block this section
                    if deferred_post:
                        deferred_post.pop(0)()
                    drip(unit)
                    pv = score_exp_pv(
                        h, qh, half, ktp, pvt[h], f"{qh}_{half}_{h}_{ktp}",
                        first=(unit == 0),
                    )
                    # defer each PV past the next unit's scores so it
                    # never blocks them in the in-order PE stream
                    deferred_pv.append(pv)
                    if len(deferred_pv) > 6:
                        deferred_pv.pop(0)()
                    unit += 1
                for pv in deferred_pv:
                    pv()
                deferred_pv = []
                if not last:
                    parts = {h: normalize_parts(h, qh, half, pvt[h]) for h in heads}
                    deferred_post = [parts[h][0] for h in heads] + [
                        parts[h][1] for h in heads
                    ]
        # tail: final section (qh1,half1) normalized in 128-column quarters so
        # each out-proj tile (12..15) starts as soon as its columns are ready
        partsA = normalize_parts(0, 1, 1, pvt[0], pieces=2, tail=True)
        partsB = normalize_parts(1, 1, 1, pvt[1], pieces=2, tail=True)
        partsA[0]()
        partsB[0]()
        for half_t in range(2):
            partsA[1 + half_t]()
            partsB[1 + half_t]()
            emit_op(12 + 2 * half_t)
            emit_op(13 + 2 * half_t)

        pv_psum.release()
        sc_psum.release()
        pj_psum.release()
        ob_pool.release()
        rc_pool.release()
        u_pool.release()
        hstb_pool.release()

    nc.compile()
    return nc


def _get_nc():
    global _CACHED_NC
    if _CACHED_NC is None:
        _CACHED_NC = _build_nc()
    return _CACHED_NC


def _fold_cape(W, P):
    """W @ blockdiag(P) for 4x4 P repeated along channels: exact CAPE fold."""
    d = W.shape[1]
    W4 = W.reshape(W.shape[0], d // 4, 4)
    return np.einsum("cik,kj->cij", W4, P, optimize=True).reshape(W.shape[0], d)


def _klayout(W):
    # [1280, cols] -> [128, KT*cols] with ktile-major free dim
    cols = W.shape[1]
    return np.ascontiguousarray(
        W.reshape(KT, 128, cols).transpose(1, 0, 2).reshape(128, KT * cols)
    )


def _pairlayout(W, dtype):
    # [1280, cols] -> [128, KP*2*cols] channel-pair-major (r = kt parity)
    cols = W.shape[1]
    return np.ascontiguousarray(
        W.reshape(KP, 2, 128, cols).transpose(2, 0, 1, 3).reshape(128, KP * 2 * cols),
        dtype=dtype,
    )


def _prep_in_maps(hidden_states, p_out, p_out_inv, Wq, Wk, Wv, Wo):
    scale = HD ** -0.5
    hs2 = np.ascontiguousarray(hidden_states.reshape(S, D), dtype=np.float32)
    hs_sw = np.ascontiguousarray(np.concatenate([hs2[L:], hs2[:L]], axis=0))

    # 0.5 on the q side cancels the doubled broadcast-pair score matmul
    Wq_eff = [
        _fold_cape(Wq, p_out_inv[0, t]).astype(np.float32) * (scale * 0.5)
        for t in range(2)
    ]
    Wk_eff = [_fold_cape(Wk, p_out[0, t]).astype(np.float32) for t in range(2)]

    hstb_by_par = {}
    for par, h in ((0, hs2), (1, hs_sw)):
        hT = np.asarray(h.T, dtype=NPF8)  # [1280, 2048]
        cm = hT.reshape(KT, 128, 8, 256).transpose(2, 1, 0, 3)
        hstb_by_par[par] = np.ascontiguousarray(cm).reshape(8 * 128, KT * 256)

    in_maps = []
    for c in range(N_CORES):
        m, par = divmod(c, 2)
        if par == 0:
            heads = (5 * m, 5 * m + 1, 5 * m + 2)
            frames = (0, 1)
        else:
            heads = (5 * m + 3, 5 * m + 4, 5 * m + 2)
            frames = (1, 0)
        hA, hB, hC = heads

        def hcols(W, h):
            return W[:, h * HD : (h + 1) * HD]

        wg_blocks = []
        wk_blocks = []
        for t in frames:
            q_eff, k_eff = Wq_eff[t], Wk_eff[t]
            g0 = np.concatenate([hcols(q_eff, hA), hcols(q_eff, hB)], axis=1)
            g1 = np.concatenate([hcols(k_eff, hA), hcols(k_eff, hB)], axis=1)
            g2 = np.concatenate([hcols(q_eff, hC), hcols(k_eff, hC)], axis=1)
            wg_blocks.append(_klayout(g0))
            wg_blocks.append(_klayout(g2))
            wk_blocks.append(_pairlayout(g1, NPF8))
        wg = np.ascontiguousarray(np.concatenate(wg_blocks, axis=1), dtype=NPBF16)
        wk8 = np.ascontiguousarray(np.concatenate(wk_blocks, axis=1), dtype=NPF8)

        wv_cols = np.concatenate(
            [hcols(Wv, hA), hcols(Wv, hB), hcols(Wv, hC), np.zeros((D, 64), np.float32)],
            axis=1,
        )
        wv8 = _pairlayout(wv_cols, NPF8)

        def hrows(W, h):
            return W[h * HD : (h + 1) * HD, :]

        wo_plane0 = np.concatenate([hrows(Wo, hA), np.zeros((64, D), np.float32)], axis=0)
        wo_plane1 = np.concatenate([hrows(Wo, hC), np.zeros((64, D), np.float32)], axis=0)
        wo8 = np.ascontiguousarray(
            np.concatenate([wo_plane0, wo_plane1], axis=1), dtype=NPF8
        )
        wob8 = np.ascontiguousarray(hrows(Wo, hB), dtype=NPF8)

        in_maps.append(
            {
                "hstb": hstb_by_par[par],
                "wg": wg,
                "wk8": wk8,
                "wv8": wv8,
                "wo8": wo8,
                "wob8": wob8,
            }
        )
    return in_maps


def kernel(hidden_states, p_out, p_out_inv, Wq, Wk, Wv, Wo, bo):
    hidden_states = np.asarray(hidden_states, dtype=np.float32)
    in_maps = _prep_in_maps(
        hidden_states,
        np.asarray(p_out, np.float32),
        np.asarray(p_out_inv, np.float32),
        np.asarray(Wq, np.float32),
        np.asarray(Wk, np.float32),
        np.asarray(Wv, np.float32),
        np.asarray(Wo, np.float32),
    )
    nc = _get_nc()
    res = run_bass_kernel_spmd(nc, in_maps, core_ids=list(range(N_CORES)))
    acc = np.zeros((S, D), np.float32)
    for c in range(N_CORES):
        o = np.asarray(res.results[c]["out"], dtype=np.float32)
        if c % 2 == 1:
            o = np.concatenate([o[L:], o[:L]], axis=0)
        acc += o
    acc += np.asarray(bo, np.float32)[None, :]
    out = acc.reshape(2, L, D) + hidden_states.reshape(2, L, D)
    return out


# revision 69
# speedup vs baseline: 1.0105x; 1.0105x over previous
"""TRN2 Bass kernel for nn_AttentionStoreProcessor (dense transformer attention).

Full (unsharded) inputs in, full output out. Internally:
  - CAPE rotation + softmax scale folded into Wq/Wk on host (exact linear
    algebra, per-frame 4x4 block-diagonal right-multiply).
  - Balanced 2.5-head sharding: each core owns 2 full heads (A, B) and one
    half head (C, one query half). Odd cores get their hs token-halves
    swapped on host (attention is permutation-invariant over keys) so one
    SPMD program covers both half assignments; host un-swaps their output.
  - hs arrives pre-transposed from host as one fp8 [ch, tok] tensor that
    serves the q/k projections (fp8 moving x bf16 stationary weights) and
    the v projection (DoubleRow channel pairs are just [:, 2kp:2kp+2, :]).
  - q/k weights in bf16 (accuracy-critical); v projection, scores,
    probs*V and the output projection all in fp8e4m3 with DoubleRow perf
    mode (0.5 PE cycles/row). Scores broadcast the pair dim (0-stride, x2
    result folded into Wq); PV and the output projection contract real
    pairs (two kt tiles / both outT planes per matmul).
  - softmax: max-free exp (scores are O(10)); constant bias -4.5 keeps
    exp in fp8 range; denominators via a ones-column appended to V; the
    per-query reciprocal is broadcast with a K=1 matmul.
  - Activation engine runs only the exps plus a few tail copies (exp time
    is the roofline for this shard); PSUM evacuations go to DVE (gpsimd
    cannot access PSUM on real hardware). Emission order software-pipelines
    the in-order engines: projections drip between attention units on a
    due-date worklist, PV matmuls and normalize chains are deferred past
    the scores they would otherwise block, and the final query-half is
    normalized in pieces so the last out-proj tiles start immediately.
  - residual, bias and the cross-core partial-sum reduction happen on host.
"""
import numpy as np
import ml_dtypes
from contextlib import ExitStack

import concourse.bacc as bacc
import concourse.mybir as mybir
import concourse.tile as tile
from concourse.bass_utils import run_bass_kernel_spmd

F32 = mybir.dt.float32
F32R = mybir.dt.float32r
BF16 = mybir.dt.bfloat16
F8 = mybir.dt.float8e4
NPBF16 = ml_dtypes.bfloat16
NPF8 = ml_dtypes.float8_e4m3
AF = mybir.ActivationFunctionType
DR = mybir.MatmulPerfMode.DoubleRow

HEADS = 20
N_CORES = 8
S = 2048  # tokens
D = 1280  # channels
HD = 64  # head dim
L = 1024  # tokens per frame
KT = D // 128  # 10 contraction tiles for projections
KP = KT // 2  # 5 channel-pair tiles
TOKT = S // 128  # 16 token tiles
EXP_BIAS = -4.5

_CACHED_NC = None


def _build_nc():
    nc = bacc.Bacc("TRN2", debug=False, num_devices=N_CORES)

    # hs^T fp8, chunk-major: [8 token-chunks, 128, KT, 256] so each chunk
    # is one contiguous-per-partition DMA. Serves the q/k projections as fp8
    # moving data (bf16 stationary weights) and the v projection as DoubleRow
    # channel pairs via [:, ci, 2kp:2kp+2, :].
    hstb_d = nc.dram_tensor("hstb", [8 * 128, KT * 256], F8, kind="ExternalInput").ap()
    # q-side + C-head weights bf16: 4 blocks [128, KT*128] = (t0:g0,g2, t1:g0,g2)
    wg_d = nc.dram_tensor("wg", [128, 4 * KT * 128], BF16, kind="ExternalInput").ap()
    # A/B k-side weights fp8 pair layout [128, t, KP, 2, 128]
    wk_d = nc.dram_tensor("wk8", [128, 2 * KP * 2 * 128], F8, kind="ExternalInput").ap()
    # v weights fp8 pair layout [128, KP, 2, 256]
    wv_d = nc.dram_tensor("wv8", [128, KP * 2 * 256], F8, kind="ExternalInput").ap()
    # out-proj weights fp8 [128, 2, D]: plane 0 = (A|0), plane 1 = (C|0);
    # B is applied from oT1tmp with its own base-0 weights wob
    wo_d = nc.dram_tensor("wo8", [128, 2 * D], F8, kind="ExternalInput").ap()
    wob_d = nc.dram_tensor("wob8", [64, D], F8, kind="ExternalInput").ap()
    out = nc.dram_tensor("out", [S, D], BF16, kind="ExternalOutput").ap()

    out_r = out.rearrange("(n p) d -> n p d", p=128)

    with (
        tile.TileContext(nc) as tc,
        ExitStack() as ctx,
        nc.allow_low_precision(reason="fp8/bf16 used deliberately; tolerance 2e-2"),
    ):
        persist = ctx.enter_context(tc.tile_pool(name="persist", bufs=1))
        hstb_pool = tc.alloc_tile_pool(name="hstb", bufs=1)
        u_pool = tc.alloc_tile_pool(name="u", bufs=12)
        rc_pool = tc.alloc_tile_pool(name="rc", bufs=3)
        ob_pool = tc.alloc_tile_pool(name="ob", bufs=6)

        pj_psum = tc.alloc_tile_pool(name="pj", bufs=1, space="PSUM")
        sc_psum = tc.alloc_tile_pool(name="sc", bufs=2, space="PSUM")
        pv_psum = tc.alloc_tile_pool(name="pv", bufs=3, space="PSUM")

        # ---- persistent tiles ----
        ones_sb = persist.tile([128, 64], BF16, tag="ones")
        expbias = persist.tile([128, 1], F32, tag="expbias")
        # per-kt stride padded 195 -> 208: dual-fp8 ldweights requires the
        # pair-dim step to be even and 16B-aligned
        v195 = persist.tile([128, TOKT, 208], F8, tag="v195")
        QA = persist.tile([128, S], F8, tag="QA")  # rows 0:64 qA, 64:128 qB
        KA = persist.tile([128, S], F8, tag="KA")  # rows 0:64 kA, 64:128 kB
        QK2 = persist.tile([128, S], F8, tag="QK2")  # rows 0:64 qC, 64:128 kC
        QB2 = persist.tile([128, S], F8, tag="QB2")  # rows 64:128 <- qC (shifted)
        outTall = persist.tile([128, 2, S], F8, tag="outTall")
        oT1tmp = persist.tile([64, S], F8, tag="oT1tmp")
        wg_sb = persist.tile([128, 4, KT, 128], BF16, tag="wg")
        wk_sb = persist.tile([128, 2, KP, 2, 128], F8, tag="wk8")
        wv_sb = persist.tile([128, KP, 2, 256], F8, tag="wv8")
        wo_sb = persist.tile([128, 2, D], F8, tag="wo8")
        wob_sb = persist.tile([64, D], F8, tag="wob8")

        nc.gpsimd.memset(ones_sb[:], 1.0)
        nc.gpsimd.memset(expbias[:], EXP_BIAS)
        # ones columns of v_ext (col 65h+64 = 1.0); plane-1 zeros of outTall
        v195_h = v195[:, :, 0:195].rearrange("p n (h x) -> p n h x", h=3)
        nc.vector.memset(v195_h[:, :, :, 64:65], 1.0)
        nc.vector.memset(outTall[:, 1, :], 0.0)
        nc.vector.memset(outTall[64:128, 0, :], 0.0)

        # ---- input DMAs ----
        # scalar queue: weights only (all dispatched before the first exp);
        # sync queue: hs tiles, chunk-pipelined.
        def wg_dma(g):
            nc.scalar.dma_start(
                wg_sb[:, g, :, :],
                wg_d[:, g * KT * 128 : (g + 1) * KT * 128].rearrange(
                    "p (k m) -> p k m", k=KT
                ),
            )

        wg_dma(0)
        nc.scalar.dma_start(
            wk_sb[:],
            wk_d.rearrange("p (t k two m) -> p t k two m", t=2, k=KP, two=2),
        )
        nc.scalar.dma_start(
            wv_sb[:], wv_d.rearrange("p (k two m) -> p k two m", k=KP, two=2)
        )
        for g in range(1, 4):
            wg_dma(g)
        nc.scalar.dma_start(wo_sb[:], wo_d.rearrange("p (two d) -> p two d", two=2))
        nc.scalar.dma_start(wob_sb[:], wob_d)
        hstb = hstb_pool.tile([128, 8, KT, 256], F8, tag="hstb")
        hstb_src = hstb_d.rearrange("(c p) (k s) -> p c k s", p=128, k=KT)
        nc.sync.dma_start(hstb[:, 0, 0:5, :], hstb_src[:, 0, 0:5, :])
        nc.sync.dma_start(hstb[:, 0, 5:10, :], hstb_src[:, 0, 5:10, :])
        for ci in range(1, 8):
            nc.sync.dma_start(hstb[:, ci, :, :], hstb_src[:, ci, :, :])

        # ---- projection work units (emitted interleaved with attention) ----
        # one psum bank, manually double-buffered by alternating 256-col
        # halves so an item's accumulation never waits the previous item's
        # DVE evacuation (range-level deps keep the halves independent)
        proj_ps = pj_psum.tile([128, 512], F32, tag="pj")
        pj_flip = [0]

        def pj_half():
            pj_flip[0] ^= 1
            o = pj_flip[0] * 256
            return proj_ps[:, o : o + 256]

        def emit_qk(ci, g):
            t = ci // 4  # core-local frame
            qs = slice(ci * 256, (ci + 1) * 256)
            dest = (QA, KA, QK2)[g]
            pp = pj_half()
            if g == 1:
                # A/B k-side in fp8 DoubleRow over channel pairs (4x fewer
                # PE cycles; scores requantize k to fp8 anyway)
                for kp in range(KP):
                    nc.tensor.matmul(
                        pp,
                        wk_sb[:, t, kp, :, :],
                        hstb[:, ci, 2 * kp : 2 * kp + 2, :],
                        start=(kp == 0),
                        stop=(kp == KP - 1),
                        perf_mode=DR,
                    )
            else:
                for k in range(KT):
                    nc.tensor.matmul(
                        pp,
                        wg_sb[:, t * 2 + (0 if g == 0 else 1), k, :],
                        hstb[:, ci, k, :],
                        start=(k == 0),
                        stop=(k == KT - 1),
                    )
            nc.vector.tensor_copy(dest[:, qs], pp)
            if g == 2:
                # shift qC (QK2 rows 0:64) to QB2 rows 64:128 (same base as
                # kC); gpsimd queue: cheap SEQ, runs in the pre-outproj window
                nc.gpsimd.dma_start(QB2[64:128, qs], QK2[0:64, qs])

        qk_half_state = {}

        def emit_qk_half(ci, g, part):
            # bf16 q/k chunk in two 5-ktile halves so the PE insert between
            # attention units stays below one exp time
            t = ci // 4
            if part == 0:
                pp = pj_half()
                qk_half_state[(ci, g)] = pp
            else:
                pp = qk_half_state.pop((ci, g))
            for k in range(part * 5, part * 5 + 5):
                nc.tensor.matmul(
                    pp,
                    wg_sb[:, t * 3 + g, k, :],
                    hstb[:, ci, k, :],
                    start=(k == 0),
                    stop=(k == KT - 1),
                )
            if part == 1:
                qs = slice(ci * 256, (ci + 1) * 256)
                nc.vector.tensor_copy((QA, KA, QK2)[g][:, qs], pp)
                if g == 2:
                    nc.gpsimd.dma_start(QB2[64:128, qs], QK2[0:64, qs])

        def emit_v(n):
            # v projection for token tile n (fp8 DoubleRow over channel pairs)
            vp = pj_half()
            for kp in range(KP):
                nc.tensor.matmul(
                    vp,
                    hstb[:, n // 2, 2 * kp : 2 * kp + 2, (n % 2) * 128 : (n % 2) * 128 + 128],
                    wv_sb[:, kp, :, :],
                    start=(kp == 0),
                    stop=(kp == KP - 1),
                    perf_mode=DR,
                )
            nc.vector.tensor_copy(
                v195_h[:, n, :, 0:64],
                vp[:, 0:192].rearrange("p (h x) -> p h x", h=3),
            )

        def head_ops(h):
            # (kT source, rows, qT source, rows) -- both at the same base
            if h == 0:
                return KA, slice(0, 64), QA, slice(0, 64)
            if h == 1:
                return KA, slice(64, 128), QA, slice(64, 128)
            return QK2, slice(64, 128), QB2, slice(64, 128)

        def brc(ap, n):
            # insert broadcast pair dim: [64, n] -> [64, 2, n], stride 0
            return ap.unsqueeze(1).broadcast_to((64, 2, n))

        def score_exp_pv(h, qh, half, ktp, pvt, name, first=False):
            # one attention unit: two broadcast-pair score matmuls into a
            # [128,1024] psum tile, one wide exp (amortizes the ACT access
            # penalty), one DoubleRow PV matmul contracting both kt tiles
            ksrc, krows, qsrc, qrows = head_ops(h)
            qcol = qh * 1024 + half * 512
            u2 = u_pool.tile([128, 2, 512], F8, tag="u", name=f"u{name}")
            sc = sc_psum.tile([128, 1024], F32, tag="sc", name=f"sc{name}")
            if first:
                # very first unit: 256-column sub-scores so the first exp
                # needs only q-chunk 0; the chunk-1 projection is emitted
                # between the halves and hides behind the first exp
                for qsub in range(2):
                    if qsub == 1:
                        emit_qk(1, 0)
                    for r in range(2):
                        kt = 2 * ktp + r
                        nc.tensor.matmul(
                            sc[:, r * 512 + qsub * 256 : r * 512 + qsub * 256 + 256],
                            brc(ksrc[krows, kt * 128 : (kt + 1) * 128], 128),
                            brc(qsrc[qrows, qcol + qsub * 256 : qcol + qsub * 256 + 256], 256),
                            start=True,
                            stop=True,
                            perf_mode=DR,
                        )
                    nc.scalar.activation(
                        u2[:, :, qsub * 256 : (qsub + 1) * 256],
                        sc[:].rearrange("p (two n) -> p two n", two=2)[
                            :, :, qsub * 256 : (qsub + 1) * 256
                        ],
                        AF.Exp,
                        bias=expbias[:],
                    )
            else:
                for r in range(2):
                    kt = 2 * ktp + r
                    nc.tensor.matmul(
                        sc[:, r * 512 : (r + 1) * 512],
                        brc(ksrc[krows, kt * 128 : (kt + 1) * 128], 128),
                        brc(qsrc[qrows, qcol : qcol + 512], 512),
                        start=True,
                        stop=True,
                        perf_mode=DR,
                    )
                nc.scalar.activation(
                    u2[:],
                    sc[:].rearrange("p (two n) -> p two n", two=2),
                    AF.Exp,
                    bias=expbias[:],
                )

            def pv():
                nc.tensor.matmul(
                    pvt,
                    v195[:, 2 * ktp : 2 * ktp + 2, 65 * h : 65 * h + 65],
                    u2[:],
                    start=(ktp == 0),
                    stop=(ktp == TOKT // 2 - 1),
                    perf_mode=DR,
                )

            return pv

        def norm_dest(h, c0, c1):
            if h == 0:
                return outTall[0:64, 0, c0:c1]
            if h == 1:
                return oT1tmp[:, c0:c1]
            return outTall[0:64, 1, c0:c1]

        def normalize_parts(h, qh, sub, pvt, pieces=1, tail=False):
            # returns closures: [recip, then per piece: bc+mul]
            q0 = qh * 1024 + sub * 512
            nm = f"{h}_{qh}_{sub}"
            rc = rc_pool.tile([65, 512], BF16, tag="rc", name=f"rc{nm}")
            w = 512 // pieces

            def recip():
                nc.vector.reciprocal(rc[64:65, :], pvt[64:65, :])

            def piece(i):
                def fn():
                    ps = slice(i * w, (i + 1) * w)
                    bc = sc_psum.tile([64, w], F32, tag="sc", name=f"bc{nm}_{i}")
                    nc.tensor.matmul(
                        bc[:], ones_sb[64:65, :], rc[64:65, ps], start=True, stop=True
                    )
                    # HW allows only one PSUM input per tensor-tensor op
                    bcs = rc_pool.tile([64, w], F32, tag="bcs", name=f"bcs{nm}_{i}")
                    if tail:
                        nc.scalar.copy(bcs[:], bc[:])  # ACT idle past last exp
                    else:
                        nc.vector.tensor_copy(bcs[:], bc[:])
                    nc.vector.tensor_mul(
                        norm_dest(h, q0 + i * w, q0 + (i + 1) * w), pvt[0:64, ps], bcs[:]
                    )
                return fn

            return [recip] + [piece(i) for i in range(pieces)]

        def emit_op(n):
            # output projection for token tile n (one DoubleRow matmul per
            # 512-wide Wo chunk contracts both outT planes = 192 features)
            ts = slice(n * 128, (n + 1) * 128)
            ob = ob_pool.tile([128, D], BF16, tag="ob", name=f"ob{n}")
            for dc, (off, w) in enumerate(((0, 512), (512, 512), (1024, 256))):
                op = pv_psum.tile([128, 512], F32, tag="pv", name=f"op{n}_{dc}")
                nc.tensor.matmul(
                    op[:, 0:w],
                    outTall[:, :, ts],
                    wo_sb[:, :, off : off + w],
                    start=True,
                    stop=False,
                    perf_mode=DR,
                )
                nc.tensor.matmul(
                    op[:, 0:w],
                    oT1tmp[:, ts],
                    wob_sb[:, off : off + w],
                    start=False,
                    stop=True,
                )
                if n >= 12 and dc != 1:
                    # tail: ACT is past its last exp and otherwise idle
                    nc.scalar.copy(ob[:, off : off + w], op[:, 0:w])
                else:
                    nc.vector.tensor_copy(ob[:, off : off + w], op[:, 0:w])
            deng = nc.gpsimd if n >= 12 and n % 2 == 1 else nc.sync
            deng.dma_start(out_r[n], ob[:])

        # ---- interleaved emission: attention unit stream + projection drip ----
        # pending projection units with due-unit indices (due = unit index in
        # the first section's stream, before which the item must be emitted;
        # PE executes nearly in emission order, so due-dates track data needs:
        # v tiles 2k,2k+1 before PV at ktp=k; k-side chunks before their kt
        # range; q-sides/g2 tails before the sections that read them)
        pending = [
            (1, emit_qk, (1, 1)),
            (1, emit_qk, (0, 2)),
            (2, emit_qk, (1, 2)),
        ]
        # v tiles 2k,2k+1 feed PV at ktp=k (unit 3k); k-side 256-token chunk
        # ci feeds kt tiles 2ci,2ci+1 (A/B at unit 3ci, C one unit later)
        pending += [(max(3 * k + 2, 1), emit_v, (2 * k,)) for k in range(8)]
        pending += [(max(3 * k + 2, 1), emit_v, (2 * k + 1,)) for k in range(8)]
        pending += [(3 * ci - 4, emit_qk, (ci, 1)) for ci in range(2, 8)]
        pending += [(3 * ci, emit_qk, (ci, 2)) for ci in range(2, 8)]
        # q-side: chunks 2,3 are query-half1 of qh0 (due unit 24); 4..7 are qh1
        pending += [(13, emit_qk, (2, 0)), (17, emit_qk, (3, 0))]
        pending += [(26 + 3 * j, emit_qk, (4 + j, 0)) for j in range(4)]
        # out-projection tiles drip through the qh1 attention stream as their
        # outTall columns complete (tiles 0..7 after qh0, 8..11 after
        # qh1-half0); 12..15 are the tail
        pending += [(53 + 2 * j, emit_op, (j,)) for j in range(8)]
        pending += [(69 + 3 * j, emit_op, (8 + j,)) for j in range(4)]
        pending.sort(key=lambda e: e[0])
        pi = 0

        def drip(unit):
            nonlocal pi
            while pi < len(pending) and pending[pi][0] <= unit:
                _, fn, args = pending[pi]
                fn(*args)
                pi += 1

        # PE warm-up: the tensor engine reaches full clock only after ~3us
        # of continuous execution. While the first hs/weight DMAs are in
        # flight the PE is idle anyway, so run throwaway matmuls on the
        # already-memset ones tile; if they bridge seamlessly into the first
        # projection it starts at full speed (undershoot just reverts to the
        # status quo, the scratch is never read)
        warm = sc_psum.tile([64, 64], F32, tag="sc", name="warm")
        for i in range(50):
            nc.tensor.matmul(
                warm[:], ones_sb[0:64, :], ones_sb[0:64, :], start=True, stop=True
            )

        # phase A: the minimum before the first score (qA, kA chunk 0; qA
        # needs both 256-chunks of the first 512 query columns), then the
        # C-head projections + shifts overlap the first A/B exps
        for ci, g in ((0, 0), (0, 1)):
            emit_qk(ci, g)

        unit = 0
        deferred_pv = []
        deferred_post = []
        for qh in range(2):
            heads = (0, 1, 2) if qh == 0 else (0, 1)
            for half in range(2):
                last = qh == 1 and half == 1
                pvt = {
                    h: pv_psum.tile(
                        [65, 512], F32, tag="pv", name=f"pv{qh}_{half}_{h}"
                    )
                    for h in heads
                }
                # C lags A/B by one kt-pair so its q-shift DMA (issued by the
                # dripped g2 projection) is never on the critical PE path
                seq = []
                for ktp in range(TOKT // 2):
                    for h in heads[:2]:
                        seq.append((h, ktp))
                    if len(heads) > 2 and ktp >= 1:
                        seq.append((2, ktp - 1))
                if len(heads) > 2:
                    seq.append((2, TOKT // 2 - 1))
                for h, ktp in seq:
                    # previous section's normalize/shift closures drip
                    # one per unit so they never block this section
                    if deferred_post:
                        deferred_post.pop(0)()
                    drip(unit)
                    pv = score_exp_pv(
                        h, qh, half, ktp, pvt[h], f"{qh}_{half}_{h}_{ktp}",
                        first=(unit == 0),
                    )
                    # defer each PV past the next unit's scores so it
                    # never blocks them in the in-order PE stream
                    deferred_pv.append(pv)
                    if len(deferred_pv) > 6:
                        deferred_pv.pop(0)()
                    unit += 1
                for pv in deferred_pv:
                    pv()
                deferred_pv = []
                if not last:
                    parts = {h: normalize_parts(h, qh, half, pvt[h]) for h in heads}
                    deferred_post = [parts[h][0] for h in heads] + [
                        parts[h][1] for h in heads
                    ]
        # tail: final section (qh1,half1) normalized in 128-column quarters so
        # each out-proj tile (12..15) starts as soon as its columns are ready
        partsA = normalize_parts(0, 1, 1, pvt[0], pieces=2, tail=True)
        partsB = normalize_parts(1, 1, 1, pvt[1], pieces=2, tail=True)
        partsA[0]()
        partsB[0]()
        for half_t in range(2):
            partsA[1 + half_t]()
            partsB[1 + half_t]()
            emit_op(12 + 2 * half_t)
            emit_op(13 + 2 * half_t)

        pv_psum.release()
        sc_psum.release()
        pj_psum.release()
        ob_pool.release()
        rc_pool.release()
        u_pool.release()
        hstb_pool.release()

    nc.compile()
    return nc


def _get_nc():
    global _CACHED_NC
    if _CACHED_NC is None:
        _CACHED_NC = _build_nc()
    return _CACHED_NC


def _fold_cape(W, P):
    """W @ blockdiag(P) for 4x4 P repeated along channels: exact CAPE fold."""
    d = W.shape[1]
    W4 = W.reshape(W.shape[0], d // 4, 4)
    return np.einsum("cik,kj->cij", W4, P, optimize=True).reshape(W.shape[0], d)


def _klayout(W):
    # [1280, cols] -> [128, KT*cols] with ktile-major free dim
    cols = W.shape[1]
    return np.ascontiguousarray(
        W.reshape(KT, 128, cols).transpose(1, 0, 2).reshape(128, KT * cols)
    )


def _pairlayout(W, dtype):
    # [1280, cols] -> [128, KP*2*cols] channel-pair-major (r = kt parity)
    cols = W.shape[1]
    return np.ascontiguousarray(
        W.reshape(KP, 2, 128, cols).transpose(2, 0, 1, 3).reshape(128, KP * 2 * cols),
        dtype=dtype,
    )


def _prep_in_maps(hidden_states, p_out, p_out_inv, Wq, Wk, Wv, Wo):
    scale = HD ** -0.5
    hs2 = np.ascontiguousarray(hidden_states.reshape(S, D), dtype=np.float32)
    hs_sw = np.ascontiguousarray(np.concatenate([hs2[L:], hs2[:L]], axis=0))

    # 0.5 on the q side cancels the doubled broadcast-pair score matmul
    Wq_eff = [
        _fold_cape(Wq, p_out_inv[0, t]).astype(np.float32) * (scale * 0.5)
        for t in range(2)
    ]
    Wk_eff = [_fold_cape(Wk, p_out[0, t]).astype(np.float32) for t in range(2)]

    hstb_by_par = {}
    for par, h in ((0, hs2), (1, hs_sw)):
        hT = np.asarray(h.T, dtype=NPF8)  # [1280, 2048]
        cm = hT.reshape(KT, 128, 8, 256).transpose(2, 1, 0, 3)
        hstb_by_par[par] = np.ascontiguousarray(cm).reshape(8 * 128, KT * 256)

    in_maps = []
    for c in range(N_CORES):
        m, par = divmod(c, 2)
        if par == 0:
            heads = (5 * m, 5 * m + 1, 5 * m + 2)
            frames = (0, 1)
        else:
            heads = (5 * m + 3, 5 * m + 4, 5 * m + 2)
            frames = (1, 0)
        hA, hB, hC = heads

        def hcols(W, h):
            return W[:, h * HD : (h + 1) * HD]

        wg_blocks = []
        wk_blocks = []
        for t in frames:
            q_eff, k_eff = Wq_eff[t], Wk_eff[t]
            g0 = np.concatenate([hcols(q_eff, hA), hcols(q_eff, hB)], axis=1)
            g1 = np.concatenate([hcols(k_eff, hA), hcols(k_eff, hB)], axis=1)
            g2 = np.concatenate([hcols(q_eff, hC), hcols(k_eff, hC)], axis=1)
            wg_blocks.append(_klayout(g0))
            wg_blocks.append(_klayout(g2))
            wk_blocks.append(_pairlayout(g1, NPF8))
        wg = np.ascontiguousarray(np.concatenate(wg_blocks, axis=1), dtype=NPBF16)
        wk8 = np.ascontiguousarray(np.concatenate(wk_blocks, axis=1), dtype=NPF8)

        wv_cols = np.concatenate(
            [hcols(Wv, hA), hcols(Wv, hB), hcols(Wv, hC), np.zeros((D, 64), np.float32)],
            axis=1,
        )
        wv8 = _pairlayout(wv_cols, NPF8)

        def hrows(W, h):
            return W[h * HD : (h + 1) * HD, :]

        wo_plane0 = np.concatenate([hrows(Wo, hA), np.zeros((64, D), np.float32)], axis=0)
        wo_plane1 = np.concatenate([hrows(Wo, hC), np.zeros((64, D), np.float32)], axis=0)
        wo8 = np.ascontiguousarray(
            np.concatenate([wo_plane0, wo_plane1], axis=1), dtype=NPF8
        )
        wob8 = np.ascontiguousarray(hrows(Wo, hB), dtype=NPF8)

        in_maps.append(
            {
                "hstb": hstb_by_par[par],
                "wg": wg,
                "wk8": wk8,
                "wv8": wv8,
                "wo8": wo8,
                "wob8": wob8,
            }
        )
    return in_maps


def kernel(hidden_states, p_out, p_out_inv, Wq, Wk, Wv, Wo, bo):
    hidden_states = np.asarray(hidden_states, dtype=np.float32)
    in_maps = _prep_in_maps(
        hidden_states,
        np.asarray(p_out, np.float32),
        np.asarray(p_out_inv, np.float32),
        np.asarray(Wq, np.float32),
        np.asarray(Wk, np.float32),
        np.asarray(Wv, np.float32),
        np.asarray(Wo, np.float32),
    )
    nc = _get_nc()
    res = run_bass_kernel_spmd(nc, in_maps, core_ids=list(range(N_CORES)))
    acc = np.zeros((S, D), np.float32)
    for c in range(N_CORES):
        o = np.asarray(res.results[c]["out"], dtype=np.float32)
        if c % 2 == 1:
            o = np.concatenate([o[L:], o[:L]], axis=0)
        acc += o
    acc += np.asarray(bo, np.float32)[None, :]
    out = acc.reshape(2, L, D) + hidden_states.reshape(2, L, D)
    return out
